# revision 8
# baseline (speedup 1.0000x reference)
"""GRU (nn_RNN_34024730919141) Trainium2 Bass kernel.

Math (per reference):
    x  = embed[input]                              [B, T, E]
    gi = x @ W_ih.T + b_ih                         [B, T, 3H]
    per step: gh = h @ W_hh.T + b_hh
              r = sigmoid(gi_r + gh_r); z = sigmoid(gi_z + gh_z)
              n = tanh(gi_n + r * gh_n)
              h = (1-z)*n + z*h
    out = (log_softmax(h_final @ W_out.T + b_out), h_final)

Key transforms:
  * embed-lookup + input projection fold into one [V, 3H] table
    (embed @ W_ih.T + b_ih, with b_hh folded in for the r,z gates);
    per-step gi_t = tbl[token] is staged host-side into a gate-major
    stream [T, 128, 6, B_shard] fp16 that the device DMAs contiguously
    (same HBM bytes a device gather would read; the Q7 extended-isa
    gather ucode is not available in this image).
  * Hidden state kept transposed hT [H, B_shard] in fp16; recurrent matmul
    gh.T = W_hhT.T @ hT on the PE; gi_r/gi_z added into PSUM via
    identity-matmuls so sigmoid reads PSUM directly.
  * n-gate: t = (gh_n + b_hh_n) * r via one scalar_tensor_tensor.
  * batch split in 2 halves per core so consecutive timesteps pipeline
    across engines (PE / ACT / DVE).
  * 8 cores pure data-parallel over batch (512 rows each).

The tiny epilogue (W_out projection + log_softmax, 0.02% of FLOPs) runs on
host in fp32.
"""

import numpy as np

# ---- problem dims (hardcoded per spec) ------------------------------------
B, T, E, H, O, V = 4096, 140, 128, 256, 3, 401
G3 = 3 * H              # 768 gate rows
NCHUNK = G3 // 128      # 6 gate partition-tiles (0,1=r  2,3=z  4,5=n)
NCORES = 8
BS = B // NCORES        # 512 batch rows per core
NHALF = 2
HALF = BS // NHALF      # 256
GSTEPS = 4              # timesteps per gi-stream DMA
NG = (T + GSTEPS - 1) // GSTEPS

_NC_CACHE = {}


VARIANT = {"d_on_pool": False}


def _build_nc(t_steps, repeats=1, tgroups=None):
    """Build + compile the single-core Bass/Tile program (SPMD across cores).

    repeats > 1 wraps the whole scan in a hardware For_i loop and
    tgroups limits the distinct gi groups stored in DRAM (the scan cycles
    through them) -- both used only for differential timing so the host->
    device transfer is tiny and constant while device work scales with
    repeats.
    """
    import contextlib

    import concourse.bacc as bacc
    import concourse.bass as bass
    import concourse.mybir as mybir
    import concourse.tile as tile

    dt = mybir.dt
    AF = mybir.ActivationFunctionType
    ALU = mybir.AluOpType

    ngroups = (t_steps + GSTEPS - 1) // GSTEPS
    stored_groups = ngroups if tgroups is None else min(tgroups, ngroups)

    nc = bacc.Bacc(
        "TRN2",
        target_bir_lowering=False,
        debug=False,
        enable_asserts=False,
        num_devices=1,
    )

    # gi stream: [group][partition][step-in-group][chunk][batch]
    gih = nc.dram_tensor(
        "gi", [stored_groups, 128, GSTEPS, NCHUNK, BS], dt.float16,
        kind="ExternalInput"
    )
    whh = nc.dram_tensor("whh", [H, G3], dt.float16, kind="ExternalInput")
    iden = nc.dram_tensor("iden", [128, 128], dt.float16, kind="ExternalInput")
    bnn = nc.dram_tensor("bnn", [128, 2], dt.float32, kind="ExternalInput")
    h0t = nc.dram_tensor("h0t", [H, BS], dt.float16, kind="ExternalInput")
    hout = nc.dram_tensor("hout", [H, BS], dt.float16, kind="ExternalOutput")

    with tile.TileContext(nc) as tc:
        with (
            tc.tile_pool(name="const", bufs=1) as constp,
            tc.tile_pool(name="gi", bufs=3) as gip,
            tc.tile_pool(name="work", bufs=3) as workp,
            tc.tile_pool(name="state", bufs=1) as statep,
            tc.tile_pool(name="psum", bufs=1, space=bass.MemorySpace.PSUM) as psump,
        ):
            whh_sb = constp.tile([128, 2, G3], dt.float16, tag="whh")
            iden_sb = constp.tile([128, 128], dt.float16, tag="iden")
            bnn_sb = constp.tile([128, 2], dt.float32, tag="bnn")
            hT = [
                statep.tile([128, 2, HALF], dt.float16, tag=f"hT{hf}", name=f"hT{hf}")
                for hf in range(NHALF)
            ]

            nc.sync.dma_start(whh_sb[:, 0, :], whh[0:128, :])
            nc.sync.dma_start(whh_sb[:, 1, :], whh[128:256, :])
            nc.sync.dma_start(iden_sb[:], iden[:, :])
            nc.sync.dma_start(bnn_sb[:], bnn[:, :])
            for hf in range(NHALF):
                for k in range(2):
                    nc.sync.dma_start(
                        hT[hf][:, k, :],
                        h0t[k * 128 : (k + 1) * 128, hf * HALF : (hf + 1) * HALF],
                    )

            def scan_body():
                gi = None
                for t in range(t_steps):
                    g, off = divmod(t, GSTEPS)
                    if off == 0:
                        gi = gip.tile(
                            [128, GSTEPS, NCHUNK, BS], dt.float16, tag="gi", name="gi_t"
                        )
                        nc.sync.dma_start(
                            gi[:, :, :, :], gih[g % stored_groups, :, :, :, :]
                        )
                    for hf in range(NHALF):
                        col = hf * HALF
                        h_t = hT[hf]
                        # ---- PE: gates into PSUM ----------------------------
                        rz = psump.tile(
                            [128, 4, HALF], dt.float32, tag=f"rz{hf}", name=f"rz{hf}"
                        )
                        nn = psump.tile(
                            [128, 2, HALF], dt.float32, tag=f"nn{hf}", name=f"nn{hf}"
                        )
                        for gt in range(4):
                            for k in range(2):
                                nc.tensor.matmul(
                                    rz[:, gt, :],
                                    whh_sb[:, k, gt * 128 : (gt + 1) * 128],
                                    h_t[:, k, :],
                                    start=(k == 0),
                                    stop=False,
                                )
                            nc.tensor.matmul(
                                rz[:, gt, :],
                                iden_sb[:, :],
                                gi[:, off, gt, col : col + HALF],
                                start=False,
                                stop=True,
                            )
                        for nt in range(2):
                            for k in range(2):
                                nc.tensor.matmul(
                                    nn[:, nt, :],
                                    whh_sb[:, k, (4 + nt) * 128 : (5 + nt) * 128],
                                    h_t[:, k, :],
                                    start=(k == 0),
                                    stop=(k == 1),
                                )
                        # ---- ACT: sigmoid over r,z --------------------------
                        s = workp.tile(
                            [128, 4, HALF], dt.float16, tag=f"s{hf}", name=f"s{hf}"
                        )
                        nc.scalar.activation(s[:, :, :], rz[:, :, :], AF.Sigmoid)
                        # ---- DVE: n-gate chain ------------------------------
                        tt = workp.tile(
                            [128, 2, HALF], dt.float16, tag=f"t{hf}", name=f"t{hf}"
                        )
                        for nt in range(2):
                            nc.vector.scalar_tensor_tensor(
                                tt[:, nt, :],
                                nn[:, nt, :],
                                bnn_sb[:, nt : nt + 1],
                                s[:, nt, :],
                                op0=ALU.add,
                                op1=ALU.mult,
                            )
                        u = workp.tile(
                            [128, 2, HALF], dt.float16, tag=f"u{hf}", name=f"u{hf}"
                        )
                        nc.vector.tensor_tensor(
                            u[:, :, :],
                            tt[:, :, :],
                            gi[:, off, 4:6, col : col + HALF],
                            op=ALU.add,
                        )
                        nt_s = workp.tile(
                            [128, 2, HALF], dt.float16, tag=f"n{hf}", name=f"ns{hf}"
                        )
                        nc.scalar.activation(nt_s[:, :, :], u[:, :, :], AF.Tanh)
                        # ---- DVE: h' = n + z*(h-n) --------------------------
                        d = workp.tile(
                            [128, 2, HALF], dt.float16, tag=f"d{hf}", name=f"d{hf}"
                        )
                        d_eng = nc.gpsimd if VARIANT["d_on_pool"] else nc.vector
                        d_eng.tensor_tensor(
                            d[:, :, :], h_t[:, :, :], nt_s[:, :, :], op=ALU.subtract
                        )
                        e = workp.tile(
                            [128, 2, HALF], dt.float16, tag=f"e{hf}", name=f"e{hf}"
                        )
                        nc.vector.tensor_tensor(
                            e[:, :, :], s[:, 2:4, :], d[:, :, :], op=ALU.mult
                        )
                        nc.vector.tensor_tensor(
                            h_t[:, :, :], nt_s[:, :, :], e[:, :, :], op=ALU.add
                        )

            if repeats > 1:
                with tc.For_i(0, repeats, 1):
                    scan_body()
            else:
                scan_body()

            for hf in range(NHALF):
                for k in range(2):
                    nc.sync.dma_start(
                        hout[k * 128 : (k + 1) * 128, hf * HALF : (hf + 1) * HALF],
                        hT[hf][:, k, :],
                    )

    nc.compile()
    return nc


def get_nc(t_steps=T, repeats=1, tgroups=None):
    key = (t_steps, repeats, tgroups, VARIANT["d_on_pool"])
    if key not in _NC_CACHE:
        _NC_CACHE[key] = _build_nc(t_steps, repeats, tgroups)
    return _NC_CACHE[key]


# --------------------------------------------------------------------------
# host-side packing
# --------------------------------------------------------------------------

def _prepare_inputs(input, hidden, embed, W_ih, W_hh, b_ih, b_hh, t_steps):
    """Build the 8 per-core input maps."""
    input = np.asarray(input)
    hidden = np.asarray(hidden, dtype=np.float32)
    embed = np.asarray(embed, dtype=np.float32)
    W_ih = np.asarray(W_ih, dtype=np.float32)
    W_hh = np.asarray(W_hh, dtype=np.float32)
    b_ih = np.asarray(b_ih, dtype=np.float32)
    b_hh = np.asarray(b_hh, dtype=np.float32)

    ngroups = (t_steps + GSTEPS - 1) // GSTEPS
    tpad = ngroups * GSTEPS

    # gate table: embed @ W_ih.T + b_ih, with b_hh folded in for r,z
    tblf = embed @ W_ih.T + b_ih
    tblf[:, : 2 * H] += b_hh[: 2 * H]
    tbl = tblf.astype(np.float16)

    whh = np.ascontiguousarray(W_hh.T).astype(np.float16)          # [H, 3H]
    iden = np.eye(128, dtype=np.float16)
    bnn = np.ascontiguousarray(b_hh[2 * H :].reshape(2, 128).T).astype(np.float32)

    toks = input.astype(np.int64)

    in_maps = []
    for c in range(NCORES):
        tc_ = toks[c * BS : (c + 1) * BS, :t_steps]                 # [BS, t]
        if tpad != t_steps:
            tc_ = np.pad(tc_, ((0, 0), (0, tpad - t_steps)))
        # gi_c[b, t, g] -> gi stream [g(group), p, s, c, b]
        gi_c = tbl[tc_]                                             # [BS, tpad, 768]
        gi_c = gi_c.reshape(BS, ngroups, GSTEPS, NCHUNK, 128)
        gi_c = np.ascontiguousarray(gi_c.transpose(1, 4, 2, 3, 0))  # [ng,128,GS,6,BS]

        h0t = np.ascontiguousarray(
            hidden[c * BS : (c + 1) * BS, :].T
        ).astype(np.float16)                                        # [H, BS]

        in_maps.append(
            {
                "gi": gi_c,
                "whh": whh,
                "iden": iden,
                "bnn": bnn,
                "h0t": h0t,
            }
        )
    return in_maps


_LAST_RESULTS = {"exec_time_ns": None, "mean_exec_time_ns": None}


def run_device(inputs, t_steps=T, trace=False, trace_kwargs=None):
    """Run the sharded kernel on 8 cores; returns h_final [B, H] fp32."""
    from concourse.bass_utils import run_bass_kernel_spmd

    nc = get_nc(t_steps)
    in_maps = _prepare_inputs(
        inputs["input"], inputs["hidden"], inputs["embed"], inputs["W_ih"],
        inputs["W_hh"], inputs["b_ih"], inputs["b_hh"], t_steps,
    )
    res = run_bass_kernel_spmd(
        nc,
        in_maps,
        core_ids=list(range(NCORES)),
        trace=trace,
        **(trace_kwargs or {}),
    )
    _LAST_RESULTS["exec_time_ns"] = res.exec_time_ns
    _LAST_RESULTS["mean_exec_time_ns"] = res.mean_exec_time_ns
    h_final = np.empty((B, H), dtype=np.float32)
    for c in range(NCORES):
        h_final[c * BS : (c + 1) * BS, :] = res.results[c]["hout"].astype(np.float32).T
    return h_final


def kernel(input, hidden, embed, W_ih, W_hh, b_ih, b_hh, W_out, b_out):
    inputs = dict(
        input=input, hidden=hidden, embed=embed, W_ih=W_ih, W_hh=W_hh,
        b_ih=b_ih, b_hh=b_hh,
    )
    h_final = run_device(inputs, T)
    W_out = np.asarray(W_out, dtype=np.float32)
    b_out = np.asarray(b_out, dtype=np.float32)
    logits = h_final @ W_out.T + b_out
    m = logits.max(axis=1, keepdims=True)
    lse = m + np.log(np.exp(logits - m).sum(axis=1, keepdims=True))
    return (logits - lse).astype(np.float32), h_final
